# revision 1
# baseline (speedup 1.0000x reference)
"""Single attention head (B=8, S=2048, D=768, H=12) on 8 TRN2 NeuronCores.

Data-parallel over batch (1 element/core). Host prep is layout only:
  - per-batch permutation placing masked-in keys first (key extent compacts
    from 2048 to T_pad ~ 1152),
  - x transposed to (D, S) and split into fp16 hi/lo limbs (x scaled by 16
    so limb residuals stay in fp16 normal range),
  - weights packed [Wk | Wq/sqrt(H) | Wv] at 32-aligned columns, scaled by
    64 and split into fp16 limbs (products carry 2^10; descaled on egress),
  - mask converted to an additive fp16 bias row (0 / -60000).

Device pipeline per core (all matmuls fp16-rate; fp32 matmuls on TRN2 are
~4x slower because the compiler splits them into hi/lo passes):
  1. qkvT (96p, S) = 3 limb passes of W^T @ xT accumulated in PSUM.
  2. pass A (s-part): scores_hi = qh.kh + bias via an extra contraction row
     -> DVE reduce_max -> row max m (only needs +-85 accuracy).
  3. pass B (t-part): qh.kh + ql.kh + qh.kl + bias - m, all five terms as
     38 stacked contraction rows in ONE matmul -> ACT exp from PSUM -> fp16
     pT tiles.
  4. out_aug = [v | 1]^T @ pT (softmax denominator free in row 12)
     -> transpose 128-blocks back to s-part, multiply by reciprocal of sums.
"""

import math
import os

import numpy as np

B, S, D, H = 8, 2048, 768, 12
N_CORES = 8
BIAS_NEG = -60000.0
DS = 2.0 ** -10   # descale after limb matmuls (x*16, w*64)


def _build(nc_mod, T_pad):
    bass, mybir, tile, bacc = nc_mod
    f32 = mybir.dt.float32
    f16 = mybir.dt.float16
    AF = mybir.ActivationFunctionType
    OP = mybir.AluOpType
    X = mybir.AxisListType.X

    NT = T_pad // 128           # t tiles
    HALF = T_pad // 2           # A-pass half extent (multiple of 64)
    NCH = 4                     # s chunks
    SCH = S // NCH              # 512
    STC = SCH // 128            # s tiles per chunk = 4

    nc = bacc.Bacc("TRN2", target_bir_lowering=False, debug=False,
                   num_devices=N_CORES)

    xh_ext = nc.dram_tensor("xh", [D, S], f16, kind="ExternalInput")
    xl_ext = nc.dram_tensor("xl", [D, S], f16, kind="ExternalInput")
    w_ext = nc.dram_tensor("w", [D, 192], f16, kind="ExternalInput")
    bias_ext = nc.dram_tensor("biasrow", [1, T_pad], f16, kind="ExternalInput")
    out_ext = nc.dram_tensor("out", [128, 256], f32, kind="ExternalOutput")

    from concourse.masks import make_identity

    with tile.TileContext(nc) as tc:
        with tc.tile_pool(name="sb", bufs=1) as sb, \
             tc.tile_pool(name="tmp", bufs=2) as tmp, \
             tc.tile_pool(name="pt", bufs=4) as ptp:

            xh = sb.tile([128, 6, S], f16)
            xl = sb.tile([128, 6, S], f16)
            w = sb.tile([128, 6, 192], f16)   # [wh(96) | wl(96)] per k-tile
            # kTb rows: 0-11 kh, 12 bias, 13 -1, 14-25 kh dup, 26-37 kl,
            #           64-101 replica of 0-37
            kTb = sb.tile([128, T_pad], f16)
            # rhsB rows: 0-11 qh, 12 +1, 13 m, 14-25 ql, 26-37 qh dup,
            #           64-101 replica of 0-37
            rhsB = sb.tile([128, S], f16)
            qlst = sb.tile([12, S], f16)      # DVE staging for ql
            klst = sb.tile([12, T_pad], f16)  # DVE staging for kl
            vTsb = sb.tile([12, T_pad], f32)
            vaug = sb.tile([128, NT, 16], f16)
            ident = sb.tile([128, 128], f32)
            maxh = sb.tile([128, 2, 16], f32)
            maxc = sb.tile([128, 16], f32)
            negst = sb.tile([1, 2, SCH], f16)
            vaccs = sb.tile([16, 2, SCH], f32)
            outsb = sb.tile([128, 16, 16], f32)

            nc.gpsimd.memset(kTb[:, :], -1.0)   # row 13 stays -1.0
            nc.gpsimd.memset(rhsB[:, :], 1.0)   # rows 12 / 76 stay +1.0
            nc.gpsimd.memset(vaccs[:], 0.0)     # rows 13-15 stay 0

            nc.sync.dma_start(w[:], w_ext.ap().rearrange(
                "(ko p) m -> p ko m", p=128))
            nc.sync.dma_start(kTb[12:13, :], bias_ext.ap())
            xhr = xh_ext.ap().rearrange("(ko p) s -> p ko s", p=128)
            xlr = xl_ext.ap().rearrange("(ko p) s -> p ko s", p=128)
            for c in range(NCH):
                cs = slice(c * SCH, (c + 1) * SCH)
                nc.sync.dma_start(xh[:, :, cs], xhr[:, :, cs])
                nc.sync.dma_start(xl[:, :, cs], xlr[:, :, cs])

            make_identity(nc, ident[:])
            nc.gpsimd.memset(vaug[:, :, 12:16], 0.0)
            nc.gpsimd.memset(vaug[:, :, 12:13], 1.0)

            ncov = (T_pad + SCH - 1) // SCH
            with tc.tile_pool(name="projp", bufs=1, space="PSUM") as projp:
                qkv = projp.tile([96, S], f32)
                for c in range(NCH):
                    cs = slice(c * SCH, (c + 1) * SCH)
                    for ps in range(3):   # wh*xh, wl*xh, wh*xl
                        wsl = slice(96, 192) if ps == 1 else slice(0, 96)
                        xin = xl if ps == 2 else xh
                        for ko in range(6):
                            nc.tensor.matmul(
                                qkv[0:96, cs], w[:, ko, wsl], xin[:, ko, cs],
                                start=(ps == 0 and ko == 0),
                                stop=(ps == 2 and ko == 5))
                    # qh / ql egress for this chunk (+1024 descale)
                    nc.scalar.mul(rhsB[0:12, cs], qkv[32:44, cs], DS)
                    nc.vector.scalar_tensor_tensor(
                        qlst[:, cs], qkv[32:44, cs], DS, rhsB[0:12, cs],
                        op0=OP.mult, op1=OP.subtract)
                    nc.sync.dma_start(rhsB[14:26, cs], qlst[:, cs])
                    nc.sync.dma_start(rhsB[26:38, cs], rhsB[0:12, cs])
                    # base-64 replica rows (A lhsT + B rhs packing)
                    nc.sync.dma_start(rhsB[64:76, cs], rhsB[0:12, cs])
                    nc.sync.dma_start(rhsB[78:90, cs], qlst[:, cs])
                    nc.sync.dma_start(rhsB[90:102, cs], rhsB[0:12, cs])
                    if c == ncov - 1:
                        nc.scalar.mul(kTb[0:12, :], qkv[0:12, 0:T_pad], DS)
                        nc.vector.scalar_tensor_tensor(
                            klst[:], qkv[0:12, 0:T_pad], DS, kTb[0:12, :],
                            op0=OP.mult, op1=OP.subtract)
                        nc.scalar.mul(vTsb[:], qkv[64:76, 0:T_pad], DS)
                        nc.sync.dma_start(kTb[14:26, :], kTb[0:12, :])
                        nc.sync.dma_start(kTb[26:38, :], klst[:])
                        nc.sync.dma_start(kTb[64:102, :], kTb[0:38, :])

            with tc.tile_pool(name="Ap", bufs=2, space="PSUM") as Ap, \
                 tc.tile_pool(name="smp", bufs=3, space="PSUM") as smp, \
                 tc.tile_pool(name="vap", bufs=1, space="PSUM") as vap:

                for j in range(NT):
                    trv = smp.tile([128, 512], f32, tag="sm")
                    nc.tensor.transpose(
                        trv[0:128, 0:12], vTsb[:, j * 128:(j + 1) * 128],
                        ident[0:12, 0:12])
                    nc.vector.tensor_copy(vaug[:, j, 0:12], trv[0:128, 0:12])

                def emit_A_mm(c):
                    for pr in range(STC // 2):
                        st0 = c * STC + 2 * pr
                        st1 = st0 + 1
                        s0 = slice(st0 * 128, (st0 + 1) * 128)
                        s1 = slice(st1 * 128, (st1 + 1) * 128)
                        for h in range(2):
                            t0 = Ap.tile([128, 1024], f32, tag="A")
                            t1 = Ap.tile([128, 1024], f32, tag="A")
                            base = h * HALF
                            for off in range(0, HALF, 512):
                                n = min(512, HALF - off)
                                tsl = slice(base + off, base + off + n)
                                nc.tensor.matmul(
                                    t0[:, off:off + n], rhsB[0:13, s0],
                                    kTb[0:13, tsl], start=True, stop=True,
                                    tile_position=(0, 0))
                                nc.tensor.matmul(
                                    t1[:, off:off + n], rhsB[64:77, s1],
                                    kTb[64:77, tsl], start=True, stop=True,
                                    tile_position=(64, 0))
                            nc.vector.reduce_max(
                                maxh[:, h, st0:st0 + 1], t0[:, 0:HALF], axis=X)
                            nc.vector.reduce_max(
                                maxh[:, h, st1:st1 + 1], t1[:, 0:HALF], axis=X)
                    c4 = slice(c * STC, (c + 1) * STC)
                    nc.vector.tensor_max(
                        maxc[:, c4], maxh[:, 0, c4], maxh[:, 1, c4])

                def emit_negm(c):
                    mt = smp.tile([128, 512], f32, tag="sm")
                    for k in range(STC):
                        st = c * STC + k
                        nc.tensor.transpose(
                            mt[0:1, k * 128:(k + 1) * 128],
                            maxc[:, st:st + 1], ident[:])
                    cs = slice(c * SCH, (c + 1) * SCH)
                    nc.scalar.copy(negst[:, c % 2, :], mt[0:1, 0:SCH])
                    nc.sync.dma_start(rhsB[13:14, cs], negst[:, c % 2, :])
                    nc.sync.dma_start(rhsB[77:78, cs], negst[:, c % 2, :])

                def emit_B(c):
                    cs = slice(c * SCH, (c + 1) * SCH)
                    vacc = vap.tile([13, SCH], f32)
                    for jp in range((NT + 1) // 2):
                        j0, j1 = 2 * jp, 2 * jp + 1
                        bt = []
                        for j, rb, tp in ((j0, 0, (0, 0)), (j1, 64, (64, 0))):
                            if j >= NT:
                                continue
                            bp = smp.tile([128, 512], f32, tag="sm")
                            tsl = slice(j * 128, (j + 1) * 128)
                            nc.tensor.matmul(
                                bp[:, 0:SCH], kTb[rb:rb + 38, tsl],
                                rhsB[rb:rb + 38, cs], start=True, stop=True,
                                tile_position=tp)
                            bt.append((j, bp))
                        for j, bp in bt:
                            p = ptp.tile([128, SCH], f16, tag="p")
                            nc.scalar.activation(p[:], bp[:, 0:SCH], AF.Exp)
                            nc.tensor.matmul(
                                vacc[0:13, :], vaug[:, j, 0:13], p[:],
                                start=(j == 0), stop=(j == NT - 1))
                    nc.scalar.copy(vaccs[0:13, c % 2, :], vacc[0:13, :])

                def emit_out(c):
                    tro = smp.tile([128, 512], f32, tag="sm")
                    for k in range(STC):
                        nc.tensor.transpose(
                            tro[:, 16 * k:16 * k + 16],
                            vaccs[:, c % 2, k * 128:(k + 1) * 128],
                            ident[0:16, 0:16])
                    trr = tro[:].rearrange("p (k x) -> p k x", x=16)
                    rec = tmp.tile([128, 4], f32, tag="rec")
                    recb = tmp.tile([128, 4, 16], f32, tag="recb")
                    nc.vector.reciprocal(rec[:], trr[:, 0:4, 12])
                    nc.vector.tensor_copy(
                        recb[:], rec[:, :, None].to_broadcast([128, 4, 16]))
                    nc.vector.tensor_mul(
                        outsb[:, c * STC:(c + 1) * STC, :],
                        trr[:, 0:4, :], recb[:])

                emit_A_mm(0)
                emit_negm(0)
                for c in range(NCH):
                    if c + 1 < NCH:
                        emit_A_mm(c + 1)
                    emit_B(c)
                    if c + 1 < NCH:
                        emit_negm(c + 1)
                    if c >= 1:
                        emit_out(c - 1)
                emit_out(NCH - 1)

            nc.sync.dma_start(
                out_ext.ap(), outsb[:].rearrange("p a b -> p (a b)"))

    nc.compile()
    return nc


def kernel(x, mask, key_weight, query_weight, value_weight):
    import concourse.bass as bass
    import concourse.mybir as mybir
    import concourse.tile as tile
    from concourse import bacc, bass_utils

    x = np.asarray(x, dtype=np.float32)
    mask = np.asarray(mask)
    wk = np.asarray(key_weight, dtype=np.float32)
    wq = np.asarray(query_weight, dtype=np.float32)
    wv = np.asarray(value_weight, dtype=np.float32)

    # natural-units W, 32-aligned columns, x64 scale for fp16 limb split
    w2 = np.zeros((D, 96), dtype=np.float32)
    w2[:, 0:12] = wk
    w2[:, 32:44] = wq / math.sqrt(H)
    w2[:, 64:76] = wv
    w2 *= 64.0
    wh = w2.astype(np.float16)
    wl = (w2 - wh.astype(np.float32)).astype(np.float16)
    w_cat = np.concatenate([wh, wl], axis=1)  # (768, 192) fp16

    perms, nbs = [], []
    for b in range(B):
        m = mask[b, 0].astype(np.int64)
        perm = np.argsort(1 - m, kind="stable")
        perms.append(perm)
        nbs.append(int(m.sum()))
    T_pad = max(128, int(np.ceil(max(max(nbs), 1) / 128.0)) * 128)
    T_pad = min(T_pad, S)

    in_maps = []
    for b in range(B):
        xs = np.ascontiguousarray(x[b].T[:, perms[b]]) * 16.0
        xsh = xs.astype(np.float16)
        xsl = (xs - xsh.astype(np.float32)).astype(np.float16)
        biasrow = np.zeros((1, T_pad), dtype=np.float16)
        biasrow[0, nbs[b]:] = BIAS_NEG
        in_maps.append({"xh": xsh, "xl": xsl, "w": w_cat, "biasrow": biasrow})

    import time as _time
    _t0 = _time.time()
    print(f"[kernel] building graph, T_pad={T_pad}", flush=True)
    nc = _build((bass, mybir, tile, bacc), T_pad)
    print(f"[kernel] graph+bacc compile done in {_time.time() - _t0:.1f}s",
          flush=True)

    trace = os.environ.get("BASS_KERNEL_TRACE", "0") == "1"
    if trace:
        import sys
        import types
        from trn_agent_boot.trn_boot import _ntff_profile_via_ctypes
        hook = _ntff_profile_via_ctypes("/opt/axon/libaxon_pjrt.so")
        m = types.ModuleType("antenv.axon_hooks")
        m.get_axon_ntff_profile_hook = lambda: hook
        sys.modules["antenv.axon_hooks"] = m
        bass_utils.upload_artifacts = lambda tmpdir: "local://" + tmpdir

    res = bass_utils.run_bass_kernel_spmd(
        nc, in_maps, core_ids=list(range(N_CORES)), trace=trace)
    if trace:
        print(f"HW exec time: {res.exec_time_ns} ns", flush=True)

    out = np.empty((B, S, H), dtype=np.float32)
    for b in range(B):
        o = res.results[b]["out"].reshape(128, 16, 16)[:, :, :H]
        out[b, perms[b], :] = o.transpose(1, 0, 2).reshape(S, H)
    return out



# revision 10
# speedup vs baseline: 1.2691x; 1.2691x over previous
"""Single attention head (B=8, S=2048, D=768, H=12) on 8 TRN2 NeuronCores.

Data-parallel over batch (1 element/core). Host prep is layout only:
  - per-batch permutation placing masked-in keys first (key extent compacts
    from 2048 to T_pad ~ 1152),
  - x transposed to (D, S), scaled by 16, split into fp16 hi/lo limbs, and
    stored chunk-blocked so each DMA line is one long contiguous run,
  - weights packed [Wk | Wq/sqrt(H) | Wv] at 32-aligned columns, scaled by
    64, split into fp16 limbs,
  - mask converted to a NEGATED additive bias row (0 / +60000).

Device pipeline per core (single score pass; ACT-paced):
  1. qkvT (96p, chunk) = 3 limb passes of W^T @ xT in PSUM, t-range chunks
     first so K/V are ready early.  Egress negated q rows (hi/lo) and
     positive k rows (hi/lo) + v as fp16.
  2. scores: per 128-row s-tile, ONE 37-row matmul streams all T_pad cols:
     rows (-qh|-ql|-qh)x(kh|kh|kl) + 1 x negbias -> PSUM holds -s.
  3. DVE reduce_min -> -m; ACT exp(-1*psum + (-m)) with per-partition bias
     -> p tile fp16 (softmax numerator, unnormalized).
  4. DMA-xbar transpose p -> pT (t-partition layout), off the PE/DVE/ACT.
  5. AV: 4-way column-tiled matmuls [v|1]^T @ pT accumulate per col-group;
     PSUM -> DRAM raw; host sums col groups, divides by the ones-row, and
     inverts the permutation.
"""

import math
import os

import numpy as np

B, S, D, H = 8, 2048, 768, 12
N_CORES = 8
BIAS_NEG = 60000.0       # negated additive mask bias
DS = 2.0 ** -10          # descale after limb matmuls (x*16, w*64)


def _ranges(lo, hi, step):
    out = []
    c = lo
    while c < hi:
        out.append((c, min(c + step, hi)))
        c = min(c + step, hi)
    return out


def _build(nc_mod, T_pad):
    bass, mybir, tile, bacc = nc_mod
    f32 = mybir.dt.float32
    f16 = mybir.dt.float16
    AF = mybir.ActivationFunctionType
    OP = mybir.AluOpType
    X = mybir.AxisListType.X

    NT = T_pad // 128            # t tiles
    NS = S // 128                # s tiles (16)
    tchunks = _ranges(0, T_pad, 512)         # proj chunks covering key range
    schunks = _ranges(T_pad, S, 512)         # proj chunks, query-only range
    chunks = tchunks + schunks
    tsplits = _ranges(0, T_pad, 512)         # per-s-tile score sub-matmuls

    nc = bacc.Bacc("TRN2", target_bir_lowering=False, debug=False,
                   num_devices=N_CORES)

    xh_ext = nc.dram_tensor("xh", [128, 6 * S], f16, kind="ExternalInput")
    xl_ext = nc.dram_tensor("xl", [128, 6 * S], f16, kind="ExternalInput")
    w_ext = nc.dram_tensor("w", [128, 6 * 192], f16, kind="ExternalInput")
    nbias_ext = nc.dram_tensor("nbias", [2, T_pad], f16, kind="ExternalInput")
    out_ext = nc.dram_tensor("out", [128, S], f32, kind="ExternalOutput")

    with tile.TileContext(nc) as tc:
        with tc.tile_pool(name="sb", bufs=1) as sb, \
             tc.tile_pool(name="pp", bufs=3) as pp:

            xh = sb.tile([128, 6, S], f16)
            xl = sb.tile([128, 6, S], f16)
            w = sb.tile([128, 6, 192], f16)    # [wh(96) | wl(96)] per k-tile
            # kTb rows: 0-11 kh, 12-23 kh dup, 24-35 kl, 36 negbias
            kTb = sb.tile([128, T_pad], f16)
            # rhsB rows: 0-11 -qh, 12-23 -ql, 24-35 -qh dup, 36 = +1
            rhsB = sb.tile([128, S], f16)
            qlst = sb.tile([12, S], f16)       # -ql staging (partition 0-11)
            klst = sb.tile([12, T_pad], f16)   # kl staging
            vsb = sb.tile([16, T_pad], f16)    # v rows + ones row 12
            vaug = sb.tile([128, NT, 16], f16)
            negm = sb.tile([128, NS], f32)     # -rowmax per s-tile
            osb = sb.tile([128, 2, 512], f32)  # AV psum egress staging
            pT = sb.tile([128, NT, S], f16)    # transposed softmax numerators
            wrm = sb.tile([1, 1], f32)

            # ACT exp table preload at t~0 (one dummy exp)
            nc.gpsimd.memset(wrm[:], 0.0)
            nc.scalar.activation(wrm[:], wrm[:], AF.Exp)

            nc.gpsimd.memset(rhsB[:, :], 1.0)   # row 36 stays +1
            nc.gpsimd.memset(vsb[:, :], 0.0)

            nc.sync.dma_start(w[:], w_ext.ap().rearrange(
                "p (ko m) -> p ko m", m=192))
            nc.sync.dma_start(kTb[36:37, :], nbias_ext.ap()[0:1, :])
            nc.sync.dma_start(vsb[12:13, :], nbias_ext.ap()[1:2, :])
            off = 0
            for (c0, c1) in chunks:
                lc = c1 - c0
                for ext, dst in ((xh_ext, xh), (xl_ext, xl)):
                    src = ext.ap()[:, off:off + 6 * lc].rearrange(
                        "p (ko s) -> p ko s", s=lc)
                    nc.sync.dma_start(dst[:, :, c0:c1], src)
                off += 6 * lc

            it_last = len(tchunks) - 1
            with tc.tile_pool(name="scp", bufs=2, space="PSUM") as scp:
                with tc.tile_pool(name="projp", bufs=2, space="PSUM") as projp:
                    for ci, (c0, c1) in enumerate(chunks):
                        lc = c1 - c0
                        ps = projp.tile([96, 512], f32, tag="ps")
                        for pas in range(3):   # wh*xh, wl*xh, wh*xl
                            wsl = slice(96, 192) if pas == 1 else slice(0, 96)
                            xin = xl if pas == 2 else xh
                            for ko in range(6):
                                nc.tensor.matmul(
                                    ps[0:96, 0:lc], w[:, ko, wsl],
                                    xin[:, ko, c0:c1],
                                    start=(pas == 0 and ko == 0),
                                    stop=(pas == 2 and ko == 5))
                        # negated q rows (hi then lo residual)
                        nc.vector.tensor_scalar_mul(
                            rhsB[0:12, c0:c1], ps[32:44, 0:lc], -DS)
                        nc.vector.scalar_tensor_tensor(
                            qlst[:, c0:c1], ps[32:44, 0:lc], -DS,
                            rhsB[0:12, c0:c1], op0=OP.mult, op1=OP.subtract)
                        if ci <= it_last:
                            nc.vector.tensor_scalar_mul(
                                kTb[0:12, c0:c1], ps[0:12, 0:lc], DS)
                            nc.vector.scalar_tensor_tensor(
                                klst[:, c0:c1], ps[0:12, 0:lc], DS,
                                kTb[0:12, c0:c1], op0=OP.mult,
                                op1=OP.subtract)
                            nc.vector.tensor_scalar_mul(
                                vsb[0:12, c0:c1], ps[64:76, 0:lc], DS)
                        if ci == it_last:
                            # k-side row dups + v transpose, cols 0..T_pad
                            nc.sync.dma_start(kTb[12:24, :], kTb[0:12, :])
                            nc.sync.dma_start(kTb[24:36, :], klst[:, :])
                            nc.sync.dma_start_transpose(vaug[:], vsb[:])
                            # q-side shift/dup for cols 0..T_pad
                            nc.sync.dma_start(
                                rhsB[12:24, 0:T_pad], qlst[:, 0:T_pad])
                            nc.sync.dma_start(
                                rhsB[24:36, 0:T_pad], rhsB[0:12, 0:T_pad])
                        if ci == len(chunks) - 1 and T_pad < S:
                            nc.sync.dma_start(
                                rhsB[12:24, T_pad:S], qlst[:, T_pad:S])
                            nc.sync.dma_start(
                                rhsB[24:36, T_pad:S], rhsB[0:12, T_pad:S])

                # scores + softmax numerators, ACT-paced pipeline
                for i in range(NS):
                    s0 = i * 128
                    sp = scp.tile([128, 1536], f32, tag="sc")
                    for (t0, t1) in tsplits:
                        nc.tensor.matmul(
                            sp[:, t0:t1], rhsB[0:37, s0:s0 + 128],
                            kTb[0:37, t0:t1], start=True, stop=True)
                    nc.vector.tensor_reduce(
                        negm[:, i:i + 1], sp[:, 0:T_pad], axis=X,
                        op=OP.min)
                    p = pp.tile([128, T_pad], f16, tag="p")
                    nc.scalar.activation(
                        p[:], sp[:, 0:T_pad], AF.Exp,
                        bias=negm[:, i:i + 1], scale=-1.0)
                    nc.sync.dma_start_transpose(
                        pT[:, :, s0:s0 + 128], p[:])

                # AV: 4-way column-tiled, accumulate per col-group
                with tc.tile_pool(name="avp", bufs=2, space="PSUM") as avp:
                    for c in range(S // 512):
                        cs = slice(c * 512, (c + 1) * 512)
                        vac = avp.tile([128, 512], f32, tag="av")
                        for j in range(NT):
                            g = j % 4
                            nc.tensor.matmul(
                                vac[32 * g:32 * g + 13, :],
                                vaug[:, j, 0:13], pT[:, j, cs],
                                start=(j < 4), stop=(j + 4 >= NT),
                                tile_position=(0, 32 * g))
                        nc.vector.tensor_copy(osb[:, c % 2, :], vac[:])
                        nc.sync.dma_start(
                            out_ext.ap()[:, cs], osb[:, c % 2, :])

    nc.compile()
    return nc


def kernel(x, mask, key_weight, query_weight, value_weight):
    import concourse.bass as bass
    import concourse.mybir as mybir
    import concourse.tile as tile
    from concourse import bacc, bass_utils

    x = np.asarray(x, dtype=np.float32)
    mask = np.asarray(mask)
    wk = np.asarray(key_weight, dtype=np.float32)
    wq = np.asarray(query_weight, dtype=np.float32)
    wv = np.asarray(value_weight, dtype=np.float32)

    # natural-units W, 32-aligned columns, x64 scale for fp16 limb split
    w2 = np.zeros((D, 96), dtype=np.float32)
    w2[:, 0:12] = wk
    w2[:, 32:44] = wq / math.sqrt(H)
    w2[:, 64:76] = wv
    w2 *= 64.0
    wh = w2.astype(np.float16)
    wl = (w2 - wh.astype(np.float32)).astype(np.float16)
    w_cat = np.concatenate([wh, wl], axis=1)          # (768, 192) fp16
    w_host = np.ascontiguousarray(
        w_cat.reshape(6, 128, 192).transpose(1, 0, 2).reshape(128, 6 * 192))

    perms, nbs = [], []
    for b in range(B):
        m = mask[b, 0].astype(np.int64)
        perm = np.argsort(1 - m, kind="stable")
        perms.append(perm)
        nbs.append(int(m.sum()))
    T_pad = max(128, int(np.ceil(max(max(nbs), 1) / 128.0)) * 128)
    T_pad = min(T_pad, S)

    chunks = _ranges(0, T_pad, 512) + _ranges(T_pad, S, 512)

    in_maps = []
    for b in range(B):
        xs = np.ascontiguousarray(x[b].T[:, perms[b]]) * 16.0
        xsh = xs.astype(np.float16)
        xsl = (xs - xsh.astype(np.float32)).astype(np.float16)

        def blocked(a):   # (768, S) -> chunk-blocked (128, 6*S)
            a = a.reshape(6, 128, S).transpose(1, 0, 2)   # (128, 6, S)
            return np.concatenate(
                [np.ascontiguousarray(a[:, :, c0:c1]).reshape(128, -1)
                 for (c0, c1) in chunks], axis=1)

        nb_row = np.zeros((2, T_pad), dtype=np.float16)
        nb_row[0, nbs[b]:] = BIAS_NEG
        nb_row[1, :] = 1.0
        in_maps.append({"xh": blocked(xsh), "xl": blocked(xsl),
                        "w": w_host, "nbias": nb_row})

    import time as _time
    _t0 = _time.time()
    print(f"[kernel] building graph, T_pad={T_pad}", flush=True)
    nc = _build((bass, mybir, tile, bacc), T_pad)
    print(f"[kernel] graph+bacc compile done in {_time.time() - _t0:.1f}s",
          flush=True)

    trace = os.environ.get("BASS_KERNEL_TRACE", "0") == "1"
    if trace:
        import sys
        import types
        from trn_agent_boot.trn_boot import _ntff_profile_via_ctypes
        hook = _ntff_profile_via_ctypes("/opt/axon/libaxon_pjrt.so")
        m = types.ModuleType("antenv.axon_hooks")
        m.get_axon_ntff_profile_hook = lambda: hook
        sys.modules["antenv.axon_hooks"] = m
        bass_utils.upload_artifacts = lambda tmpdir: "local://" + tmpdir

    res = bass_utils.run_bass_kernel_spmd(
        nc, in_maps, core_ids=list(range(N_CORES)), trace=trace)
    if trace:
        print(f"HW exec time: {res.exec_time_ns} ns", flush=True)

    out = np.empty((B, S, H), dtype=np.float32)
    for b in range(B):
        r = res.results[b]["out"]                     # (128, S) f32
        aug = (r[0:13] + r[32:45] + r[64:77] + r[96:109]).astype(np.float64)
        o = (aug[0:12] / aug[12][None, :]).T          # (S, H)
        out[b, perms[b], :] = o.astype(np.float32)
    return out
